# revision 1
# baseline (speedup 1.0000x reference)
"""PrRoIPool2D (precise ROI pooling) Trainium2 kernel — 8-core SPMD.

Strategy ("fused banded sweep"):
  out[r,c,p,q] = sum_{h,w} F[b_r,c,h,w] * Iy[r,p,h] * Ix[r,q,w]
The (Iy ⊗ Ix) basis is banded: bin (r,p) touches only a ~5-row window of h.
Host packs, per core (= one feature batch), a basis tensor B whose columns are
(r,p,q) output columns sorted by h-window start; for each 2-row h-chunk k the
alive columns form one contiguous interval [LO_k, HI_k).  The device then runs
one matmul per (chunk, c-half, psum-bank-piece) with the features as stationary
weights, PSUM-accumulating straight into the final output columns (per-element
has_written semantics make first-write overwrite, later writes add).  No
intermediate tensor is ever evacuated — only the final [256, COLS] output.

Everything F-dependent runs on device; the host only does O(R*(H+W)) coordinate
preprocessing (tent-basis integrals, sorting, packing) and output unpermutation.
"""

import numpy as np
import ml_dtypes

POOLED = 7
SCALE = 0.5
N, C, H, W = 8, 256, 56, 56
NCORES = 8
CHUNK_H = 2
NCHUNK = H // CHUNK_H          # 28
KDIM = CHUNK_H * W             # 112 (payload rows)
KPAD = 128                     # device K rows (padded for fast weight load)
SIM_SAFE = False               # True: split MMs for CoreSim's uniformity assert
BANK = 512                     # fp32 elements per PSUM bank
BF16 = ml_dtypes.bfloat16

_kernel_cache = {}
LAST_RESULTS = None            # BassKernelResults stash for test harnesses


def _tent_integral(start, end, n):
    i = np.arange(n, dtype=np.float64)
    a = np.clip(start[..., None] - i, -1.0, 1.0)
    b = np.clip(end[..., None] - i, -1.0, 1.0)

    def G(t):
        return np.where(t <= 0.0, 0.5 * (t + 1.0) ** 2, 1.0 - 0.5 * (1.0 - t) ** 2)

    return G(b) - G(a)


def _host_prep(features, rois):
    """Build per-core packed device inputs + unpack metadata."""
    R = rois.shape[0]
    batch = rois[:, 0].astype(np.int32)
    x1 = rois[:, 1].astype(np.float64) * SCALE
    y1 = rois[:, 2].astype(np.float64) * SCALE
    x2 = rois[:, 3].astype(np.float64) * SCALE
    y2 = rois[:, 4].astype(np.float64) * SCALE
    bw = (x2 - x1) / POOLED
    bh = (y2 - y1) / POOLED
    pw = np.arange(POOLED, dtype=np.float64)
    xs = x1[:, None] + pw * bw[:, None]
    ys = y1[:, None] + pw * bh[:, None]
    Ix = _tent_integral(xs, xs + bw[:, None], W)       # [R,7,W]
    Iy = _tent_integral(ys, ys + bh[:, None], H)       # [R,7,H]
    area = bw * bh
    scl = np.where(area > 0, 1.0 / np.maximum(area, 1e-12), 0.0)
    Iy_s = Iy * scl[:, None, None]

    core_rois = [np.nonzero(batch == c)[0] for c in range(NCORES)]
    Rmax = max(len(ix) for ix in core_rois)
    NGRP = Rmax * POOLED
    COLS = NGRP * POOLED
    NBANK = (COLS + BANK - 1) // BANK
    COLS_PAD = NBANK * BANK

    # per-core sorted group windows
    meta = []
    for c in range(NCORES):
        idx = core_rois[c]
        wins = []
        for rg in idx:
            for p in range(POOLED):
                nz = np.nonzero(Iy_s[rg, p] != 0)[0]
                lo, hi = (int(nz[0]), int(nz[-1])) if len(nz) else (0, 0)
                wins.append((lo, hi, rg, p))
        nd = (Rmax - len(idx)) * POOLED
        for dnum in range(nd):
            hf = (dnum * H) // max(nd, 1)
            wins.append((hf, hf, -1, -1))
        wins.sort(key=lambda t: (t[0], t[1]))
        meta.append(wins)

    # per-chunk alive interval (union over cores), in group units
    LO = np.full(NCHUNK, NGRP, dtype=np.int64)
    HI = np.zeros(NCHUNK, dtype=np.int64)
    for c in range(NCORES):
        wins = meta[c]
        lo_arr = np.array([w[0] for w in wins])
        hi_arr = np.array([w[1] for w in wins])
        for k in range(NCHUNK):
            h0, h1 = CHUNK_H * k, CHUNK_H * k + CHUNK_H - 1
            alive = np.nonzero((lo_arr <= h1) & (hi_arr >= h0))[0]
            if len(alive):
                LO[k] = min(LO[k], alive[0])
                HI[k] = max(HI[k], alive[-1] + 1)
    active = HI > 0
    LOc, HIc = LO * POOLED, HI * POOLED

    offs = np.zeros(NCHUNK + 1, dtype=np.int64)
    for k in range(NCHUNK):
        offs[k + 1] = offs[k] + (int(HIc[k] - LOc[k]) if active[k] else 0)
    NB = int(offs[-1])

    # pack B (bf16) per core: B[(dh,w), packed_col]
    B = np.zeros((NCORES, KDIM, NB), dtype=np.float32)
    IxT = Ix.transpose(0, 2, 1)                        # [R, W, 7]
    for c in range(NCORES):
        wins = meta[c]
        for k in range(NCHUNK):
            if not active[k]:
                continue
            for g in range(int(LO[k]), int(HI[k])):
                wlo, whi, rg, p = wins[g]
                if rg < 0:
                    continue
                cb = int(offs[k]) + (g * POOLED - int(LOc[k]))
                for dh in range(CHUNK_H):
                    h = CHUNK_H * k + dh
                    if wlo <= h <= whi:
                        B[c, dh * W:(dh + 1) * W, cb:cb + POOLED] = (
                            Iy_s[rg, p, h] * IxT[rg]
                        )
    B = np.pad(B, ((0, 0), (0, KPAD - KDIM), (0, 0))).astype(BF16)

    # features per core, chunk-major transposed: FT[(dh,w), k*C + cc]
    f = features.astype(np.float32)                    # [N,C,H,W]
    # [N, C, k, dh, w] -> [N, dh, w, k, C]
    ft = f.reshape(N, C, NCHUNK, CHUNK_H, W).transpose(0, 3, 4, 2, 1)
    FT = np.pad(ft.reshape(N, KDIM, NCHUNK * C),
                ((0, 0), (0, KPAD - KDIM), (0, 0))).astype(BF16)

    return dict(B=B, FT=FT, offs=offs, LOc=LOc.astype(int), HIc=HIc.astype(int),
                active=active, meta=meta, Rmax=Rmax, COLS=COLS,
                COLS_PAD=COLS_PAD, NBANK=NBANK, NB=NB, R=R)


def shape_cols(LOc, HIc, active):
    return max(int(HIc[k]) for k in range(NCHUNK) if active[k])


def _build_bass(shape_key):
    """Build + compile the SPMD Bass program for given packing metadata."""
    NB, COLS_PAD, NBANK, LOc, HIc, active_t, offs = shape_key
    LOc, HIc, active, offs = list(LOc), list(HIc), list(active_t), list(offs)

    import concourse.bass as bass  # noqa: F401
    import concourse.tile as tile
    from concourse import bacc, mybir

    nc = bacc.Bacc("TRN2", target_bir_lowering=False, debug=False,
                   enable_asserts=False, num_devices=NCORES)
    bf = mybir.dt.bfloat16
    f32 = mybir.dt.float32
    ft_ap = nc.dram_tensor("ft", [KPAD, NCHUNK * C], bf, kind="ExternalInput").ap()
    b_ap = nc.dram_tensor("bb", [KPAD, NB], bf, kind="ExternalInput").ap()
    COLS = shape_cols(LOc, HIc, active)
    out_ap = nc.dram_tensor("out", [C, COLS], f32, kind="ExternalOutput").ap()

    kact = [k for k in range(NCHUNK) if active[k]]
    # last chunk touching each bank (per-bank stop flag)
    last_k = {}
    for k in kact:
        for bk in range(LOc[k] // BANK, (HIc[k] - 1) // BANK + 1):
            last_k[bk] = k

    with tile.TileContext(nc) as tc:
        with (
            tc.tile_pool(name="ftp", bufs=1) as ftp,
            tc.tile_pool(name="bp", bufs=1) as bp,
            tc.tile_pool(name="pp", bufs=8, space="PSUM") as pp,
            tc.tile_pool(name="op", bufs=2) as op,
        ):
            ft_sb = ftp.tile([KPAD, NCHUNK * C], bf)
            b_sb = bp.tile([KPAD, NB], bf)
            # split input DMAs so early chunks' matmuls can start sooner
            NSPLIT = 4
            for s in range(NSPLIT):
                k0, k1 = (NCHUNK * s) // NSPLIT, (NCHUNK * (s + 1)) // NSPLIT
                nc.sync.dma_start(ft_sb[:, k0 * C:k1 * C], ft_ap[:, k0 * C:k1 * C])
                o0, o1 = offs[k0], offs[k1]
                if o1 > o0:
                    nc.scalar.dma_start(b_sb[:, o0:o1], b_ap[:, o0:o1])

            for m in range(2):
                ptiles = [pp.tile([128, BANK], f32, tag="bank", name=f"pt{m}_{i}") for i in range(NBANK)]
                # cols written so far per bank (has_written high-water mark);
                # -1 = bank untouched.  Intervals are monotone, so each new
                # matmul piece splits into an all-accumulate part (< mark) and
                # an all-fresh part (>= mark) — keeps sim's uniformity assert
                # happy and matches per-element HW semantics.
                whi = [-1] * NBANK
                for k in kact:
                    lo, hi, ob = LOc[k], HIc[k], offs[k]
                    lhsT = ft_sb[:, k * C + m * 128: k * C + (m + 1) * 128]
                    for bk in range(lo // BANK, (hi - 1) // BANK + 1):
                        s = max(lo, bk * BANK)
                        e = min(hi, (bk + 1) * BANK)
                        is_last = k == last_k[bk]
                        if whi[bk] < 0:
                            pieces = [(s, e, True)]
                        elif SIM_SAFE:
                            pieces = []
                            if s < whi[bk]:
                                pieces.append((s, min(e, whi[bk]), False))
                            if e > whi[bk]:
                                pieces.append((max(s, whi[bk]), e, False))
                        else:
                            pieces = [(s, e, False)]
                        for pi, (ps, pe, st) in enumerate(pieces):
                            nc.tensor.matmul(
                                ptiles[bk][:, ps - bk * BANK: pe - bk * BANK],
                                lhsT=lhsT,
                                rhs=b_sb[:, ob + ps - lo: ob + pe - lo],
                                start=st,
                                stop=is_last and pi == len(pieces) - 1,
                            )
                        whi[bk] = max(whi[bk], e)
                out_sb = op.tile([128, COLS], f32)
                for bk in range(NBANK):
                    w = min(BANK, COLS - bk * BANK)
                    dst = out_sb[:, bk * BANK: bk * BANK + w]
                    if bk % 2 == 0:
                        nc.vector.tensor_copy(dst, ptiles[bk][:, :w])
                    else:
                        nc.scalar.copy(dst, ptiles[bk][:, :w])
                    nc.sync.dma_start(
                        out_ap[m * 128:(m + 1) * 128, bk * BANK: bk * BANK + w],
                        dst)

    nc.compile()
    return nc


def kernel(features, rois):
    global LAST_RESULTS
    from concourse import bass_utils

    features = np.asarray(features, dtype=np.float32)
    rois = np.asarray(rois, dtype=np.float32)
    hp = _host_prep(features, rois)

    shape_key = (hp["NB"], hp["COLS_PAD"], hp["NBANK"],
                 tuple(hp["LOc"]), tuple(hp["HIc"]),
                 tuple(bool(a) for a in hp["active"]),
                 tuple(int(o) for o in hp["offs"]))
    nc = _kernel_cache.get(shape_key)
    if nc is None:
        nc = _build_bass(shape_key)
        _kernel_cache[shape_key] = nc

    in_maps = [{"ft": np.ascontiguousarray(hp["FT"][c]),
                "bb": np.ascontiguousarray(hp["B"][c])}
               for c in range(NCORES)]
    res = bass_utils.run_bass_kernel_spmd(nc, in_maps, core_ids=list(range(NCORES)))
    LAST_RESULTS = res

    # unpack: out_core[c_chan, col(g,q)] -> final[r, c_chan, p, q]
    final = np.zeros((hp["R"], C, POOLED, POOLED), dtype=np.float32)
    for c in range(NCORES):
        out = res.results[c]["out"]                    # [C, COLS]
        wins = hp["meta"][c]
        gidx = [g for g, (_, _, rg, _) in enumerate(wins) if rg >= 0]
        if not gidx:
            continue
        rgs = np.array([wins[g][2] for g in gidx])
        ps = np.array([wins[g][3] for g in gidx])
        cols = out.reshape(C, -1, POOLED)[:, gidx, :]  # [C, ngrp, 7]
        final[rgs, :, ps, :] = cols.transpose(1, 0, 2)
    return final



# revision 7
# speedup vs baseline: 1.0168x; 1.0168x over previous
"""PrRoIPool2D (precise ROI pooling) Trainium2 kernel — 8-core SPMD.

Strategy ("fused banded sweep", v2):
  out[r,c,p,q] = sum_{h,w} F[b_r,c,h,w] * Iy[r,p,h] * Ix[r,q,w]
The (Iy ⊗ Ix) basis is banded in h: bin (r,p) touches only a ~2-6 row window
(= 1-3 two-row h-chunks).  Features are sharded by batch (one batch per
core).  For each 2-row h-chunk k the device does one matmul per c-half with
the features chunk as stationary weights and a packed basis slab B as the
moving tensor, PSUM-accumulating straight into final output columns.

Column scheduling (shared across cores — the program is SPMD):
  Output columns are organized as slot cohorts: cohort a holds span-2 slots
  (alive chunks [a, a+1]) below span-3 slots (alive [a, a+2]).  At chunk k
  the alive columns are then ONE contiguous interval
  [P[k-2]+S2[k-2], P[k+1]).  Slot capacities are the max per-core demand per
  cohort/class (with pull-forward rebalancing); each core matches its (r,p)
  groups into slots whose lifetime covers the group's chunk window.  This
  cuts packed basis columns ~2x vs sorted-window interval unions.

PSUM: the alive interval is <= ~2 banks wide, so logical 512-col banks are
allocated from a rotating 8-buf PSUM pool, drained (f32 -> f16 copy + DMA
out) as soon as their last chunk is done, and the pool recycles physical
banks with auto WAR dependencies.
"""

import numpy as np
import ml_dtypes

POOLED = 7
SCALE = 0.5
N, C, H, W = 8, 256, 56, 56
NCORES = 8
CHUNK_H = 2
NCHUNK = H // CHUNK_H          # 28
KDIM = CHUNK_H * W             # 112 (contraction rows; no padding)
BANK = 512                     # fp32 elements per PSUM bank
BF16 = ml_dtypes.bfloat16
# input DMA piece boundaries (chunk indices); first pieces small so the
# matmul stream starts as soon as possible
PIECES = (0, 1, 3, 6, 11, 19, 28)
SIM_SAFE = False   # True: split matmul pieces for CoreSim's uniformity assert

_kernel_cache = {}
LAST_RESULTS = None            # BassKernelResults stash for test harnesses


def _tent_integral(start, end, n):
    i = np.arange(n, dtype=np.float64)
    a = np.clip(start[..., None] - i, -1.0, 1.0)
    b = np.clip(end[..., None] - i, -1.0, 1.0)

    def G(t):
        return np.where(t <= 0.0, 0.5 * (t + 1.0) ** 2, 1.0 - 0.5 * (1.0 - t) ** 2)

    return G(b) - G(a)


def _schedule(core_wins):
    """Slot-cohort schedule shared across cores.

    core_wins[c] = list of (k0, k1) inclusive chunk windows per group.
    Returns (S2, S3, cols) where cols[c][g] is the group's slot column.
    """
    import collections
    una = [collections.defaultdict(list) for _ in range(NCORES)]
    for c, wins in enumerate(core_wins):
        for g, (k0, k1) in enumerate(wins):
            span = k1 - k0 + 1
            assert 1 <= span <= 3, f"window span {span} unsupported"
            una[c][(k0, span)].append(g)
    S2, S3 = [0] * NCHUNK, [0] * NCHUNK
    asg2 = [[[] for _ in range(NCHUNK)] for _ in range(NCORES)]
    asg3 = [[[] for _ in range(NCHUNK)] for _ in range(NCORES)]
    for a in range(NCHUNK):
        must3 = [una[c].get((a, 3), []) for c in range(NCORES)]
        S3[a] = max(len(m) for m in must3)
        for c in range(NCORES):
            take = list(must3[c])
            una[c][(a, 3)] = []
            free = S3[a] - len(take)
            # fill spare span-3 slots with smaller windows they cover
            for key in [(a, 2), (a + 1, 2), (a, 1), (a + 1, 1), (a + 2, 1)]:
                while free > 0 and una[c].get(key):
                    take.append(una[c][key].pop())
                    free -= 1
            asg3[c][a] = take
        must2 = [una[c].get((a, 2), []) + una[c].get((a, 1), [])
                 for c in range(NCORES)]
        S2[a] = max(len(m) for m in must2)
        for c in range(NCORES):
            take = list(must2[c])
            una[c][(a, 2)] = []
            una[c][(a, 1)] = []
            free = S2[a] - len(take)
            while free > 0 and una[c].get((a + 1, 1)):
                take.append(una[c][(a + 1, 1)].pop())
                free -= 1
            asg2[c][a] = take
    for c in range(NCORES):
        assert not any(una[c].values()), "unassigned groups remain"
    P = [0] * (NCHUNK + 1)
    for a in range(NCHUNK):
        P[a + 1] = P[a] + S2[a] + S3[a]
    cols = []
    for c in range(NCORES):
        cc = {}
        for a in range(NCHUNK):
            for i, g in enumerate(asg2[c][a]):
                cc[g] = P[a] + i
            for i, g in enumerate(asg3[c][a]):
                cc[g] = P[a] + S2[a] + i
        cols.append(cc)
    return S2, S3, P, cols


def _host_prep(features, rois):
    """Build per-core packed device inputs + unpack metadata."""
    R = rois.shape[0]
    batch = rois[:, 0].astype(np.int32)
    x1 = rois[:, 1].astype(np.float64) * SCALE
    y1 = rois[:, 2].astype(np.float64) * SCALE
    x2 = rois[:, 3].astype(np.float64) * SCALE
    y2 = rois[:, 4].astype(np.float64) * SCALE
    bw = (x2 - x1) / POOLED
    bh = (y2 - y1) / POOLED
    pw = np.arange(POOLED, dtype=np.float64)
    xs = x1[:, None] + pw * bw[:, None]
    ys = y1[:, None] + pw * bh[:, None]
    Ix = _tent_integral(xs, xs + bw[:, None], W)       # [R,7,W]
    Iy = _tent_integral(ys, ys + bh[:, None], H)       # [R,7,H]
    area = bw * bh
    scl = np.where(area > 0, 1.0 / np.maximum(area, 1e-12), 0.0)
    Iy_s = Iy * scl[:, None, None]

    # per-core group list: (k0, k1) chunk windows + (rg, p, wlo, whi) payload
    core_wins, core_meta = [], []
    for c in range(NCORES):
        wins, meta = [], []
        for rg in np.nonzero(batch == c)[0]:
            for p in range(POOLED):
                nz = np.nonzero(Iy_s[rg, p] != 0)[0]
                wlo, whi = (int(nz[0]), int(nz[-1])) if len(nz) else (0, -1)
                k0, k1 = (wlo // CHUNK_H, whi // CHUNK_H) if whi >= wlo else (0, 0)
                wins.append((k0, k1))
                meta.append((rg, p, wlo, whi))
        core_wins.append(wins)
        core_meta.append(meta)

    S2, S3, P, assigns = _schedule(core_wins)
    COLS = P[-1] * POOLED
    LO = [0 if k < 2 else P[k - 2] + S2[k - 2] for k in range(NCHUNK)]
    HI = [P[min(k + 1, NCHUNK)] for k in range(NCHUNK)]
    LOc = [l * POOLED for l in LO]
    HIc = [h * POOLED for h in HI]
    offs = [0] * (NCHUNK + 1)
    for k in range(NCHUNK):
        offs[k + 1] = offs[k] + max(HIc[k] - LOc[k], 0)
    NB = offs[-1]

    # pack B (bf16) per core: B[(dh,w), packed_col]
    B = np.zeros((NCORES, KDIM, NB), dtype=np.float32)
    IxT = Ix.transpose(0, 2, 1)                        # [R, W, 7]
    for c in range(NCORES):
        cols = assigns[c]
        for g, (k0, k1) in enumerate(core_wins[c]):
            rg, p, wlo, whi = core_meta[c][g]
            if whi < wlo:
                continue
            s = cols[g]
            blk = IxT[rg]                              # [56, 7]
            for k in range(k0, k1 + 1):
                cb = offs[k] + (s - LO[k]) * POOLED
                for dh in range(CHUNK_H):
                    h = CHUNK_H * k + dh
                    if wlo <= h <= whi:
                        B[c, dh * W:(dh + 1) * W, cb:cb + POOLED] = (
                            Iy_s[rg, p, h] * blk
                        )
    B = B.astype(BF16)

    # features per core, chunk-major transposed: FT[(dh,w), k*C + cc]
    f = features.astype(np.float32)                    # [N,C,H,W]
    ft = f.reshape(N, C, NCHUNK, CHUNK_H, W).transpose(0, 3, 4, 2, 1)
    FT = ft.reshape(N, KDIM, NCHUNK * C).astype(BF16)

    return dict(B=B, FT=FT, offs=offs, LOc=LOc, HIc=HIc,
                COLS=COLS, NB=NB, R=R,
                core_meta=core_meta, assigns=assigns)


def _build_bass(shape_key):
    """Build + compile the SPMD Bass program for given packing metadata."""
    NB, COLS, LOc, HIc, offs = shape_key
    LOc, HIc, offs = list(LOc), list(HIc), list(offs)

    import concourse.bass as bass  # noqa: F401
    import concourse.tile as tile
    from concourse import bacc, mybir

    nc = bacc.Bacc("TRN2", target_bir_lowering=False, debug=False,
                   enable_asserts=False, num_devices=NCORES)
    bf = mybir.dt.bfloat16
    f16 = mybir.dt.float16
    f32 = mybir.dt.float32
    ft_ap = nc.dram_tensor("ft", [KDIM, NCHUNK * C], bf, kind="ExternalInput").ap()
    b_ap = nc.dram_tensor("bb", [KDIM, NB], bf, kind="ExternalInput").ap()
    out_ap = nc.dram_tensor("out", [C, COLS], f16, kind="ExternalOutput").ap()

    NLBANK = (COLS + BANK - 1) // BANK
    # last chunk touching each logical bank
    last_k = {}
    for k in range(NCHUNK):
        if HIc[k] > LOc[k]:
            for lb in range(LOc[k] // BANK, (HIc[k] - 1) // BANK + 1):
                last_k[lb] = k

    with tile.TileContext(nc) as tc:
        with (
            tc.tile_pool(name="ftp", bufs=1) as ftp,
            tc.tile_pool(name="bp", bufs=1) as bp,
            tc.tile_pool(name="pp", bufs=8, space="PSUM") as pp,
            tc.tile_pool(name="op", bufs=2) as op,
        ):
            ft_sb = ftp.tile([KDIM, NCHUNK * C], bf)
            b_sb = bp.tile([KDIM, NB], bf)
            # input DMAs in arrival-ordered pieces: ft on sync queue,
            # basis on scalar queue (parallel descriptor generation)
            for s in range(len(PIECES) - 1):
                k0, k1 = PIECES[s], PIECES[s + 1]
                nc.sync.dma_start(ft_sb[:, k0 * C:k1 * C], ft_ap[:, k0 * C:k1 * C])
                o0, o1 = offs[k0], offs[k1]
                if o1 > o0:
                    nc.scalar.dma_start(b_sb[:, o0:o1], b_ap[:, o0:o1])

            out_sb = {m: op.tile([128, COLS], f16, name=f"osb{m}")
                      for m in range(2)}
            for m in range(2):
                ptiles = {}
                for k in range(NCHUNK):
                    lo, hi, ob = LOc[k], HIc[k], offs[k]
                    if hi <= lo:
                        continue
                    lhsT = ft_sb[:, k * C + m * 128: k * C + (m + 1) * 128]
                    for lb in range(lo // BANK, (hi - 1) // BANK + 1):
                        s = max(lo, lb * BANK)
                        e = min(hi, (lb + 1) * BANK)
                        if lb not in ptiles:
                            ptiles[lb] = (pp.tile([128, BANK], f32, tag="bank",
                                                  name=f"pt{m}_{lb}"), -1)
                        pt, hw = ptiles[lb]
                        is_last = k == last_k[lb]
                        if hw < 0:
                            pieces = [(s, e, True)]
                        elif SIM_SAFE:
                            pieces = []
                            if s < hw:
                                pieces.append((s, min(e, hw), False))
                            if e > hw:
                                pieces.append((max(s, hw), e, False))
                        else:
                            pieces = [(s, e, False)]
                        for pi, (ps, pe, st) in enumerate(pieces):
                            nc.tensor.matmul(
                                pt[:, ps - lb * BANK: pe - lb * BANK],
                                lhsT=lhsT,
                                rhs=b_sb[:, ob + ps - lo: ob + pe - lo],
                                start=st,
                                stop=is_last and pi == len(pieces) - 1,
                            )
                        ptiles[lb] = (pt, max(hw, e))
                        # drain the bank as soon as its last chunk is done:
                        # copy (f32 -> f16) overlaps remaining matmuls and
                        # the output DMA overlaps the input stream
                        if k == last_k[lb]:
                            w = min(BANK, COLS - lb * BANK)
                            dst = out_sb[m][:, lb * BANK: lb * BANK + w]
                            if lb % 2 == 0:
                                nc.vector.tensor_copy(dst, pt[:, :w])
                            else:
                                nc.scalar.copy(dst, pt[:, :w])
                            nc.gpsimd.dma_start(
                                out_ap[m * 128:(m + 1) * 128,
                                       lb * BANK: lb * BANK + w],
                                dst)

    nc.compile()
    return nc


def kernel(features, rois):
    global LAST_RESULTS
    from concourse import bass_utils

    features = np.asarray(features, dtype=np.float32)
    rois = np.asarray(rois, dtype=np.float32)
    hp = _host_prep(features, rois)

    shape_key = (hp["NB"], hp["COLS"],
                 tuple(hp["LOc"]), tuple(hp["HIc"]), tuple(hp["offs"]))
    nc = _kernel_cache.get(shape_key)
    if nc is None:
        nc = _build_bass(shape_key)
        _kernel_cache[shape_key] = nc

    in_maps = [{"ft": np.ascontiguousarray(hp["FT"][c]),
                "bb": np.ascontiguousarray(hp["B"][c])}
               for c in range(NCORES)]
    res = bass_utils.run_bass_kernel_spmd(nc, in_maps, core_ids=list(range(NCORES)))
    LAST_RESULTS = res

    # unpack: out_core[c_chan, col(s,q)] -> final[r, c_chan, p, q]
    final = np.zeros((hp["R"], C, POOLED, POOLED), dtype=np.float32)
    for c in range(NCORES):
        out = np.asarray(res.results[c]["out"], dtype=np.float32)  # [C, COLS]
        meta = hp["core_meta"][c]
        cols = hp["assigns"][c]
        if not meta:
            continue
        rgs = np.array([m[0] for m in meta])
        ps = np.array([m[1] for m in meta])
        ss = np.array([cols[g] for g in range(len(meta))])
        blocks = out.reshape(C, -1, POOLED)[:, ss, :]   # [C, ngrp, 7]
        final[rgs, :, ps, :] = blocks.transpose(1, 0, 2)
    return final


# revision 9
# speedup vs baseline: 1.0526x; 1.0352x over previous
"""PrRoIPool2D (precise ROI pooling) Trainium2 kernel — 8-core SPMD.

Strategy ("fused banded sweep", v2):
  out[r,c,p,q] = sum_{h,w} F[b_r,c,h,w] * Iy[r,p,h] * Ix[r,q,w]
The (Iy ⊗ Ix) basis is banded in h: bin (r,p) touches only a ~2-6 row window
(= 1-3 two-row h-chunks).  Features are sharded by batch (one batch per
core).  For each 2-row h-chunk k the device does one matmul per c-half with
the features chunk as stationary weights and a packed basis slab B as the
moving tensor, PSUM-accumulating straight into final output columns.

Column scheduling (shared across cores — the program is SPMD):
  Output columns are organized as slot cohorts: cohort a holds span-2 slots
  (alive chunks [a, a+1]) below span-3 slots (alive [a, a+2]).  At chunk k
  the alive columns are then ONE contiguous interval
  [P[k-2]+S2[k-2], P[k+1]).  Slot capacities are the max per-core demand per
  cohort/class (with pull-forward rebalancing); each core matches its (r,p)
  groups into slots whose lifetime covers the group's chunk window.  This
  cuts packed basis columns ~2x vs sorted-window interval unions.

PSUM: the alive interval is <= ~2 banks wide, so logical 512-col banks are
allocated from a rotating 8-buf PSUM pool, drained (f32 -> f16 copy + DMA
out) as soon as their last chunk is done, and the pool recycles physical
banks with auto WAR dependencies.
"""

import numpy as np
import ml_dtypes

POOLED = 7
SCALE = 0.5
N, C, H, W = 8, 256, 56, 56
NCORES = 8
CHUNK_H = 2
NCHUNK = H // CHUNK_H          # 28
KDIM = CHUNK_H * W             # 112 (contraction rows; no padding)
BANK = 512                     # fp32 elements per PSUM bank
BF16 = ml_dtypes.bfloat16
# input DMA piece boundaries (chunk indices); first pieces small so the
# matmul stream starts as soon as possible
PIECES = (0, 2, 5, 10, 18, 28)
SIM_SAFE = False   # True: split matmul pieces for CoreSim's uniformity assert

_kernel_cache = {}
LAST_RESULTS = None            # BassKernelResults stash for test harnesses


def _tent_integral(start, end, n):
    i = np.arange(n, dtype=np.float64)
    a = np.clip(start[..., None] - i, -1.0, 1.0)
    b = np.clip(end[..., None] - i, -1.0, 1.0)

    def G(t):
        return np.where(t <= 0.0, 0.5 * (t + 1.0) ** 2, 1.0 - 0.5 * (1.0 - t) ** 2)

    return G(b) - G(a)


def _schedule(core_wins):
    """Slot-cohort schedule shared across cores.

    core_wins[c] = list of (k0, k1) inclusive chunk windows per group.
    Returns (S2, S3, cols) where cols[c][g] is the group's slot column.
    """
    import collections
    una = [collections.defaultdict(list) for _ in range(NCORES)]
    for c, wins in enumerate(core_wins):
        for g, (k0, k1) in enumerate(wins):
            span = k1 - k0 + 1
            assert 1 <= span <= 3, f"window span {span} unsupported"
            una[c][(k0, span)].append(g)
    S2, S3 = [0] * NCHUNK, [0] * NCHUNK
    asg2 = [[[] for _ in range(NCHUNK)] for _ in range(NCORES)]
    asg3 = [[[] for _ in range(NCHUNK)] for _ in range(NCORES)]
    for a in range(NCHUNK):
        must3 = [una[c].get((a, 3), []) for c in range(NCORES)]
        S3[a] = max(len(m) for m in must3)
        for c in range(NCORES):
            take = list(must3[c])
            una[c][(a, 3)] = []
            free = S3[a] - len(take)
            # fill spare span-3 slots with smaller windows they cover
            for key in [(a, 2), (a + 1, 2), (a, 1), (a + 1, 1), (a + 2, 1)]:
                while free > 0 and una[c].get(key):
                    take.append(una[c][key].pop())
                    free -= 1
            asg3[c][a] = take
        must2 = [una[c].get((a, 2), []) + una[c].get((a, 1), [])
                 for c in range(NCORES)]
        S2[a] = max(len(m) for m in must2)
        for c in range(NCORES):
            take = list(must2[c])
            una[c][(a, 2)] = []
            una[c][(a, 1)] = []
            free = S2[a] - len(take)
            while free > 0 and una[c].get((a + 1, 1)):
                take.append(una[c][(a + 1, 1)].pop())
                free -= 1
            asg2[c][a] = take
    for c in range(NCORES):
        assert not any(una[c].values()), "unassigned groups remain"
    P = [0] * (NCHUNK + 1)
    for a in range(NCHUNK):
        P[a + 1] = P[a] + S2[a] + S3[a]
    cols = []
    for c in range(NCORES):
        cc = {}
        for a in range(NCHUNK):
            for i, g in enumerate(asg2[c][a]):
                cc[g] = P[a] + i
            for i, g in enumerate(asg3[c][a]):
                cc[g] = P[a] + S2[a] + i
        cols.append(cc)
    return S2, S3, P, cols


def _host_prep(features, rois):
    """Build per-core packed device inputs + unpack metadata."""
    R = rois.shape[0]
    batch = rois[:, 0].astype(np.int32)
    x1 = rois[:, 1].astype(np.float64) * SCALE
    y1 = rois[:, 2].astype(np.float64) * SCALE
    x2 = rois[:, 3].astype(np.float64) * SCALE
    y2 = rois[:, 4].astype(np.float64) * SCALE
    bw = (x2 - x1) / POOLED
    bh = (y2 - y1) / POOLED
    pw = np.arange(POOLED, dtype=np.float64)
    xs = x1[:, None] + pw * bw[:, None]
    ys = y1[:, None] + pw * bh[:, None]
    Ix = _tent_integral(xs, xs + bw[:, None], W)       # [R,7,W]
    Iy = _tent_integral(ys, ys + bh[:, None], H)       # [R,7,H]
    area = bw * bh
    scl = np.where(area > 0, 1.0 / np.maximum(area, 1e-12), 0.0)
    Iy_s = Iy * scl[:, None, None]

    # per-core group list: (k0, k1) chunk windows + (rg, p, wlo, whi) payload
    core_wins, core_meta = [], []
    for c in range(NCORES):
        wins, meta = [], []
        for rg in np.nonzero(batch == c)[0]:
            for p in range(POOLED):
                nz = np.nonzero(Iy_s[rg, p] != 0)[0]
                wlo, whi = (int(nz[0]), int(nz[-1])) if len(nz) else (0, -1)
                k0, k1 = (wlo // CHUNK_H, whi // CHUNK_H) if whi >= wlo else (0, 0)
                wins.append((k0, k1))
                meta.append((rg, p, wlo, whi))
        core_wins.append(wins)
        core_meta.append(meta)

    S2, S3, P, assigns = _schedule(core_wins)
    COLS = P[-1] * POOLED
    LO = [0 if k < 2 else P[k - 2] + S2[k - 2] for k in range(NCHUNK)]
    HI = [P[min(k + 1, NCHUNK)] for k in range(NCHUNK)]
    LOc = [l * POOLED for l in LO]
    HIc = [h * POOLED for h in HI]
    offs = [0] * (NCHUNK + 1)
    for k in range(NCHUNK):
        offs[k + 1] = offs[k] + max(HIc[k] - LOc[k], 0)
    NB = offs[-1]

    # pack B (bf16) per core: B[(dh,w), packed_col]
    B = np.zeros((NCORES, KDIM, NB), dtype=np.float32)
    IxT = Ix.transpose(0, 2, 1)                        # [R, W, 7]
    for c in range(NCORES):
        cols = assigns[c]
        for g, (k0, k1) in enumerate(core_wins[c]):
            rg, p, wlo, whi = core_meta[c][g]
            if whi < wlo:
                continue
            s = cols[g]
            blk = IxT[rg]                              # [56, 7]
            for k in range(k0, k1 + 1):
                cb = offs[k] + (s - LO[k]) * POOLED
                for dh in range(CHUNK_H):
                    h = CHUNK_H * k + dh
                    if wlo <= h <= whi:
                        B[c, dh * W:(dh + 1) * W, cb:cb + POOLED] = (
                            Iy_s[rg, p, h] * blk
                        )
    B = B.astype(BF16)

    # features per core, chunk-major transposed: FT[(dh,w), k*C + cc]
    f = features.astype(np.float32)                    # [N,C,H,W]
    ft = f.reshape(N, C, NCHUNK, CHUNK_H, W).transpose(0, 3, 4, 2, 1)
    FT = ft.reshape(N, KDIM, NCHUNK * C).astype(BF16)

    return dict(B=B, FT=FT, offs=offs, LOc=LOc, HIc=HIc,
                COLS=COLS, NB=NB, R=R,
                core_meta=core_meta, assigns=assigns)


def _build_bass(shape_key):
    """Build + compile the SPMD Bass program for given packing metadata."""
    NB, COLS, LOc, HIc, offs = shape_key
    LOc, HIc, offs = list(LOc), list(HIc), list(offs)

    import concourse.bass as bass  # noqa: F401
    import concourse.tile as tile
    from concourse import bacc, mybir

    nc = bacc.Bacc("TRN2", target_bir_lowering=False, debug=False,
                   enable_asserts=False, num_devices=NCORES)
    bf = mybir.dt.bfloat16
    f16 = mybir.dt.float16
    f32 = mybir.dt.float32
    ft_ap = nc.dram_tensor("ft", [KDIM, NCHUNK * C], bf, kind="ExternalInput").ap()
    b_ap = nc.dram_tensor("bb", [KDIM, NB], bf, kind="ExternalInput").ap()
    out_ap = nc.dram_tensor("out", [C, COLS], f16, kind="ExternalOutput").ap()

    NLBANK = (COLS + BANK - 1) // BANK
    # last chunk touching each logical bank
    last_k = {}
    for k in range(NCHUNK):
        if HIc[k] > LOc[k]:
            for lb in range(LOc[k] // BANK, (HIc[k] - 1) // BANK + 1):
                last_k[lb] = k

    with tile.TileContext(nc) as tc:
        with (
            tc.tile_pool(name="ftp", bufs=1) as ftp,
            tc.tile_pool(name="bp", bufs=1) as bp,
            tc.tile_pool(name="pp", bufs=8, space="PSUM") as pp,
            tc.tile_pool(name="op", bufs=2) as op,
        ):
            ft_sb = ftp.tile([KDIM, NCHUNK * C], bf)
            b_sb = bp.tile([KDIM, NB], bf)
            # input DMAs in arrival-ordered pieces: ft on sync queue,
            # basis on scalar queue (parallel descriptor generation)
            for s in range(len(PIECES) - 1):
                k0, k1 = PIECES[s], PIECES[s + 1]
                nc.sync.dma_start(ft_sb[:, k0 * C:k1 * C], ft_ap[:, k0 * C:k1 * C])
                o0, o1 = offs[k0], offs[k1]
                if o1 > o0:
                    nc.scalar.dma_start(b_sb[:, o0:o1], b_ap[:, o0:o1])

            out_sb = {m: op.tile([128, COLS], f16, name=f"osb{m}")
                      for m in range(2)}
            # interleave both c-halves per chunk: tensor consumption then
            # tracks the DMA arrival front instead of doing two passes
            ptiles = {}
            for k in range(NCHUNK):
                lo, hi, ob = LOc[k], HIc[k], offs[k]
                if hi <= lo:
                    continue
                for m in range(2):
                    lhsT = ft_sb[:, k * C + m * 128: k * C + (m + 1) * 128]
                    for lb in range(lo // BANK, (hi - 1) // BANK + 1):
                        s = max(lo, lb * BANK)
                        e = min(hi, (lb + 1) * BANK)
                        if (m, lb) not in ptiles:
                            ptiles[(m, lb)] = (
                                pp.tile([128, BANK], f32, tag="bank",
                                        name=f"pt{m}_{lb}"), -1)
                        pt, hw = ptiles[(m, lb)]
                        is_last = k == last_k[lb]
                        if hw < 0:
                            pieces = [(s, e, True)]
                        elif SIM_SAFE:
                            pieces = []
                            if s < hw:
                                pieces.append((s, min(e, hw), False))
                            if e > hw:
                                pieces.append((max(s, hw), e, False))
                        else:
                            pieces = [(s, e, False)]
                        for pi, (ps, pe, st) in enumerate(pieces):
                            nc.tensor.matmul(
                                pt[:, ps - lb * BANK: pe - lb * BANK],
                                lhsT=lhsT,
                                rhs=b_sb[:, ob + ps - lo: ob + pe - lo],
                                start=st,
                                stop=is_last and pi == len(pieces) - 1,
                            )
                        ptiles[(m, lb)] = (pt, max(hw, e))
                        # drain the bank as soon as its last chunk is done:
                        # copy (f32 -> f16) overlaps remaining matmuls and
                        # the output DMA overlaps the input stream
                        if is_last:
                            w = min(BANK, COLS - lb * BANK)
                            dst = out_sb[m][:, lb * BANK: lb * BANK + w]
                            if m == 0:
                                nc.vector.tensor_copy(dst, pt[:, :w])
                            else:
                                nc.scalar.copy(dst, pt[:, :w])
                            nc.sync.dma_start(
                                out_ap[m * 128:(m + 1) * 128,
                                       lb * BANK: lb * BANK + w],
                                dst)

    nc.compile()
    return nc


def kernel(features, rois):
    global LAST_RESULTS
    from concourse import bass_utils

    features = np.asarray(features, dtype=np.float32)
    rois = np.asarray(rois, dtype=np.float32)
    hp = _host_prep(features, rois)

    shape_key = (hp["NB"], hp["COLS"],
                 tuple(hp["LOc"]), tuple(hp["HIc"]), tuple(hp["offs"]))
    nc = _kernel_cache.get(shape_key)
    if nc is None:
        nc = _build_bass(shape_key)
        _kernel_cache[shape_key] = nc

    in_maps = [{"ft": np.ascontiguousarray(hp["FT"][c]),
                "bb": np.ascontiguousarray(hp["B"][c])}
               for c in range(NCORES)]
    res = bass_utils.run_bass_kernel_spmd(nc, in_maps, core_ids=list(range(NCORES)))
    LAST_RESULTS = res

    # unpack: out_core[c_chan, col(s,q)] -> final[r, c_chan, p, q]
    final = np.zeros((hp["R"], C, POOLED, POOLED), dtype=np.float32)
    for c in range(NCORES):
        out = np.asarray(res.results[c]["out"], dtype=np.float32)  # [C, COLS]
        meta = hp["core_meta"][c]
        cols = hp["assigns"][c]
        if not meta:
            continue
        rgs = np.array([m[0] for m in meta])
        ps = np.array([m[1] for m in meta])
        ss = np.array([cols[g] for g in range(len(meta))])
        blocks = out.reshape(C, -1, POOLED)[:, ss, :]   # [C, ngrp, 7]
        final[rgs, :, ps, :] = blocks.transpose(1, 0, 2)
    return final
